# revision 17
# baseline (speedup 1.0000x reference)
"""Multi-head attention (S=2048, D=1024, H=16, dk=dv=64) on 8 TRN2 NeuronCores.

Sharding: head-parallel tensor parallelism. Core c owns heads {2c, 2c+1}:
  - QT/KT [128, S] (two heads stacked on partitions); V via PE-transpose of
    VT, augmented with a ones column so the ctx matmul also produces the
    softmax denominators (softmax runs over the partition axis).
  - scoresT tiles -> exp on ACT (scale=1/8 folded in) -> ctx accumulation
    trailing the scores stream inside the same chunk.
  - per s-chunk: normalize ctxT at the chunk boundary (reciprocal chain on
    DVE, broadcast via a ones-stationary PE matmul into a mega PSUM slot),
    AllGather the [128, chunk] block -> [1024, chunk], then that chunk's
    128-row slice of the output projection interleaved into later chunks.

Queue discipline: ACT (scalar) queue only issues DMA during phase 0 (idle
before the first exp); all mid-stream DMAs ride sync; collectives own the
gpsimd queue; outproj gather loads are emitted before the NEXT collective
trigger so their completion wait covers only their own AllGather.

Compute dtype: bf16 operands (host-cast), fp32 PSUM, softmax in fp32.
"""

import ml_dtypes
import numpy as np

import concourse.bass as bass
import concourse.mybir as mybir
import concourse.tile as tile
from concourse import bacc
from concourse.bass_utils import run_bass_kernel_spmd

S = 2048
D = 1024
H = 16
DK = 64
DV = 64
NCORES = 8
HPC = H // NCORES          # heads per core = 2
FW = HPC * DV              # per-core feature width = 128
P = 128                    # partitions
KT_D = D // P              # 8 contraction tiles over D
TT = S // P                # 16 tiles over t (keys)
NQ = 512                   # matmul moving free dim
CW = 512                   # s-chunk width (ctx/AG granularity)
VA = 2 * (DV + 1)          # V_aug feature width

F32 = mybir.dt.float32
BF16 = mybir.dt.bfloat16
EXPF = mybir.ActivationFunctionType.Exp
BF16NP = ml_dtypes.bfloat16

_cache = {}


def _prep_w(w):
    """[D, FW] -> [128, KT_D, FW]: row p holds all d-tiles' row p."""
    return np.ascontiguousarray(
        np.transpose(w.reshape(KT_D, P, FW), (1, 0, 2))
    ).astype(BF16NP)


def build():
    nc = bacc.Bacc(None, target_bir_lowering=False)

    enc_in = {
        x: nc.dram_tensor(f"enc{x}_t", [D, S], BF16, kind="ExternalInput")
        for x in ("q", "k", "v")
    }
    w_in = {
        n: nc.dram_tensor(n, [P, KT_D, FW], BF16, kind="ExternalInput")
        for n in ("wq", "wk", "wv", "wo")
    }
    out_t = nc.dram_tensor("outT", [FW, S], F32, kind="ExternalOutput")

    from concourse.bass import _add_dep_helper
    from concourse.masks import make_identity

    with tile.TileContext(nc) as tc:
        with (
            tc.tile_pool(name="wts", bufs=1) as wts,
            tc.tile_pool(name="encp", bufs=3) as encp,
            tc.tile_pool(name="qkv", bufs=1) as qkv,
            tc.tile_pool(name="expp", bufs=13) as expp,
            tc.tile_pool(name="catp", bufs=1) as catp,
            tc.tile_pool(name="catin", bufs=16) as catin,
            tc.tile_pool(name="misc", bufs=1) as misc,
            tc.tile_pool(name="dram", bufs=1, space="DRAM") as dram,
        ):
            rg = [list(range(NCORES))]

            # ---- warmup fodder: memset then matmul garbage to ramp PE ----
            warm_sb = misc.tile([P, NQ], BF16, tag="warmsb")
            nc.vector.memset(warm_sb[:], 0.0)

            wtiles = {}

            def load_w(name, eng):
                wt = wts.tile([P, KT_D, FW], BF16, tag=f"w_{name}", name=name)
                eng.dma_start(wt[:], w_in[name][:])
                wtiles[name] = wt

            load_w("wk", nc.sync)
            load_w("wq", nc.scalar)

            ident = wts.tile([P, P], BF16, tag="ident")
            make_identity(nc, ident)
            ones_sb = wts.tile([1, DK], BF16, tag="ones")
            nc.vector.memset(ones_sb[:], 1.0)

            # persistent SBUF state
            qt_sb = qkv.tile([P, S], BF16, tag="qt")
            kt_sb = qkv.tile([P, S], BF16, tag="kt")
            vt_sb = qkv.tile([P, S], BF16, tag="vt")
            v_aug = qkv.tile([P, TT, VA], BF16, tag="vaug")
            cat_loc = catp.tile([P, S], BF16, tag="cat")
            nc.any.memset(v_aug[:, :, DV : DV + 1], 1.0)
            nc.any.memset(v_aug[:, :, 2 * DV + 1 : 2 * DV + 2], 1.0)

            # K tiles: halves on the two HWDGE queues, straight bf16
            def load_k(dt):
                t = encp.tile([P, S], BF16, tag="bfk", bufs=5, name="bf")
                nc.sync.dma_start(
                    t[:, :1024], enc_in["k"][dt * P : (dt + 1) * P, 0:1024]
                )
                nc.scalar.dma_start(
                    t[:, 1024:], enc_in["k"][dt * P : (dt + 1) * P, 1024:]
                )
                return t

            # Q quarter tiles [128, 512], alternating queues
            gate_inst = [None]

            def load_qq(qq, dt, engs=(None, None)):
                c0 = qq * CW
                t = encp.tile([P, CW], BF16, tag="bfq", bufs=18, name="bf")
                eng = engs[dt % 2] or (nc.sync if dt % 2 == 0 else nc.scalar)
                d = eng.dma_start(
                    t[:], enc_in["q"][dt * P : (dt + 1) * P, c0 : c0 + CW]
                )
                if qq == 1 and dt == 4:
                    gate_inst[0] = d.ins
                return t

            # ---- phase 0: K, Q quarters 0+1; PE warm-up burst ----
            ps_p_cm = tc.tile_pool(name="ps_p", bufs=1, space="PSUM")
            ps_p = ps_p_cm.__enter__()
            kacc = {
                sc4: ps_p.tile([P, NQ], F32, tag=f"ka{sc4}", name=f"ka{sc4}")
                for sc4 in range(4)
            }
            wm = ps_p.tile([P, NQ], F32, tag="warm", name="wm")
            for _ in range(14):
                nc.tensor.matmul(
                    wm[:], warm_sb[:, 0:P], warm_sb[:],
                    start=True, stop=True,
                )
            for dt in range(KT_D):
                ek = load_k(dt)
                for sc4 in range(4):
                    nc.tensor.matmul(
                        kacc[sc4][:],
                        wtiles["wk"][:, dt, :],
                        ek[:, sc4 * NQ : (sc4 + 1) * NQ],
                        start=(dt == 0),
                        stop=(dt == KT_D - 1),
                    )
            load_w("wv", nc.sync)
            load_w("wo", nc.scalar)
            for sc4 in range(4):
                nc.vector.tensor_copy(
                    kt_sb[:, sc4 * NQ : (sc4 + 1) * NQ], kacc[sc4][:]
                )
            qq01 = {
                qq: ps_p.tile([P, CW], F32, tag=f"qq{qq}", name=f"qq{qq}")
                for qq in range(2)
            }
            for qq in range(2):
                for dt in range(KT_D):
                    eq = load_qq(qq, dt)
                    nc.tensor.matmul(
                        qq01[qq][:],
                        wtiles["wq"][:, dt, :],
                        eq[:],
                        start=(dt == 0),
                        stop=(dt == KT_D - 1),
                    )
                nc.vector.tensor_copy(
                    qt_sb[:, qq * CW : (qq + 1) * CW], qq01[qq][:]
                )
            ps_p_cm.__exit__(None, None, None)

            # ---- enc_v: SWDGE stream, held behind the K/Q stream ----
            ev_tiles = []
            for dt in range(KT_D):
                ev = encp.tile([P, S], BF16, tag="encv", bufs=6, name="ev")
                d = nc.gpsimd.dma_start(
                    ev[:], enc_in["v"][dt * P : (dt + 1) * P, :]
                )
                if dt == 0 and gate_inst[0] is not None:
                    _add_dep_helper(d.ins, gate_inst[0], sync=True,
                                    reason="defer enc_v behind K/Q stream")
                ev_tiles.append(ev)

            qq_pre = {}
            for qq in (2, 3):
                for dt in range(KT_D):
                    qq_pre[(qq, dt)] = load_qq(
                        qq, dt, engs=(nc.sync, nc.gpsimd)
                    )

            # ---- attention stream ----
            ps_at_cm = tc.tile_pool(name="ps_at", bufs=1, space="PSUM")
            ps_at = ps_at_cm.__enter__()
            ctx_ps = {}
            gas = {}
            exs = {}

            def scores_tt(ci, tt):
                m = ps_at.tile([P, 1024], F32, tag="mega", bufs=2, name="m")
                s0 = ci * CW
                for h in range(HPC):
                    nc.tensor.matmul(
                        m[:, h * NQ : (h + 1) * NQ],
                        kt_sb[h * DK : (h + 1) * DK, tt * P : (tt + 1) * P],
                        qt_sb[h * DK : (h + 1) * DK, s0 : s0 + NQ],
                        start=True,
                        stop=True,
                    )
                ex = expp.tile(
                    [P, 1024], BF16, tag=f"exp{tt % 2}", bufs=13, name="ex"
                )
                nc.scalar.activation(ex[:], m[:], EXPF, scale=1.0 / np.sqrt(DK))
                exs[(ci, tt)] = ex

            def ctx_op(ci, k):
                for h in range(HPC):
                    nc.tensor.matmul(
                        ctx_ps[(ci, h)][:, :],
                        v_aug[:, k, h * (DV + 1) : (h + 1) * (DV + 1)],
                        exs[(ci, k)][:, h * NQ : (h + 1) * NQ],
                        start=(k == 0),
                        stop=(k == TT - 1),
                    )

            def alloc_ctx(ci):
                for h in range(HPC):
                    ctx_ps[(ci, h)] = ps_cx.tile(
                        [DV + 1, CW], F32, tag=f"cx{h}", bufs=1,
                        name=f"cx{ci}{h}",
                    )

            recipbs = {}

            def normalize_a(ci):
                recs = []
                for h in range(HPC):
                    den = misc.tile([1, CW], F32, tag="den", bufs=2, name="den")
                    nc.vector.tensor_copy(
                        den[:], ctx_ps[(ci, h)][DV : DV + 1, :]
                    )
                    recip = misc.tile(
                        [1, CW], F32, tag="recip", bufs=2, name="recip"
                    )
                    nc.vector.reciprocal_approx_fast(recip[:], den[:])
                    recipb = misc.tile(
                        [1, CW], BF16, tag="recipb", bufs=2, name="recipb"
                    )
                    nc.vector.tensor_copy(recipb[:], recip[:])
                    recs.append(recipb)
                recipbs[ci] = recs

            def normalize_b(ci):
                c0 = ci * CW
                bc = ps_at.tile([P, 1024], F32, tag="mega", bufs=2, name="bc")
                for h in range(HPC):
                    nc.tensor.matmul(
                        bc[h * DV : (h + 1) * DV, 0:CW],
                        ones_sb[:, 0:DV],
                        recipbs[ci][h][:],
                        start=True,
                        stop=True,
                    )
                bsb = misc.tile([P, CW], F32, tag="bsb", bufs=2, name="bsb")
                nc.vector.tensor_copy(bsb[:], bc[:, 0:CW])
                for h in range(HPC):
                    nc.vector.tensor_mul(
                        cat_loc[h * DV : (h + 1) * DV, c0 : c0 + CW],
                        ctx_ps[(ci, h)][0:DV, :],
                        bsb[h * DV : (h + 1) * DV, :],
                    )
                cb = dram.tile([P, CW], BF16, tag=f"catb{ci}", name="cb")
                nc.sync.dma_start(cb[:], cat_loc[:, c0 : c0 + CW])
                ga = dram.tile([D, CW], BF16, tag=f"catall{ci}", name="ga")
                nc.gpsimd.collective_compute(
                    "AllGather",
                    mybir.AluOpType.bypass,
                    ins=[cb[:].opt()],
                    outs=[ga[:].opt()],
                    replica_groups=rg,
                )
                gas[ci] = ga

            def normalize(ci):
                normalize_a(ci)
                normalize_b(ci)

            # interleaved outproj pieces; op_dma(ci) must be emitted BEFORE
            # the NEXT chunk's collective trigger so its completion wait
            # covers only AllGather(ci).
            om_ps = {}
            op_ct = {}

            def op_dma(ci, split=False):
                for kt in range(KT_D):
                    ct = catin.tile([P, CW], BF16, tag="catkt", name="ct")
                    eng = nc.scalar if (split and kt % 2) else nc.sync
                    eng.dma_start(
                        ct[:], gas[ci][kt * P : (kt + 1) * P, :]
                    )
                    op_ct[(ci, kt)] = ct

            def op_mm(ci, kt):
                if kt == 0:
                    om_ps[ci] = ps_cx.tile(
                        [P, CW], F32, tag="aux", bufs=2, name=f"om{ci}"
                    )
                nc.tensor.matmul(
                    om_ps[ci][:],
                    wtiles["wo"][:, kt, :],
                    op_ct[(ci, kt)][:],
                    start=(kt == 0),
                    stop=(kt == KT_D - 1),
                )

            def op_fin(ci):
                c0 = ci * CW
                ob = misc.tile([P, CW], F32, tag="ob", bufs=2, name="ob")
                nc.vector.tensor_copy(ob[:], om_ps[ci][:])
                nc.sync.dma_start(out_t[:, c0 : c0 + CW], ob[:])

            # ---------- chunk 0: scores + V proj + transposes ----------
            ps_v2_cm = tc.tile_pool(name="ps_v2", bufs=1, space="PSUM")
            ps_v2 = ps_v2_cm.__enter__()
            vacc = {
                half: ps_v2.tile(
                    [P, 1024], F32, tag=f"va{half}", name=f"va{half}"
                )
                for half in range(2)
            }
            ps_cx = None
            ps_cx_cm = None
            tr_k = [0]

            def transposes(n):
                for _ in range(n):
                    k = tr_k[0]
                    if k >= TT:
                        return
                    tr_k[0] += 1
                    tp = ps_cx.tile(
                        [P, P], BF16, tag="aux", bufs=2, name="tp"
                    )
                    nc.tensor.transpose(
                        tp[:], vt_sb[:, k * P : (k + 1) * P], ident[:]
                    )
                    nc.vector.tensor_copy(v_aug[:, k, 0:DV], tp[:, 0:DV])
                    nc.vector.tensor_copy(
                        v_aug[:, k, DV + 1 : 2 * DV + 1],
                        tp[:, DV : 2 * DV],
                    )

            for tt in range(TT):
                scores_tt(0, tt)
                if 2 <= tt <= 9:
                    dt = tt - 2
                    for half in range(2):
                        for nn in range(2):
                            off = half * 1024 + nn * NQ
                            nc.tensor.matmul(
                                vacc[half][:, nn * NQ : (nn + 1) * NQ],
                                wtiles["wv"][:, dt, :],
                                ev_tiles[dt][:, off : off + NQ],
                                start=(dt == 0),
                                stop=(dt == KT_D - 1),
                            )
                if tt == 10:
                    for nn in range(4):
                        nc.vector.tensor_copy(
                            vt_sb[:, nn * NQ : (nn + 1) * NQ],
                            vacc[nn // 2][:, (nn % 2) * NQ : (nn % 2 + 1) * NQ],
                        )
                    ps_v2_cm.__exit__(None, None, None)
                    ps_cx_cm = tc.tile_pool(name="ps_cx", bufs=1, space="PSUM")
                    ps_cx = ps_cx_cm.__enter__()
                if tt >= 10:
                    transposes(3)

            # ---------- chunk 1: ctx(0) drain, AG(0), ctx(1), qq2 ----------
            nk = {0: 0, 1: 0}

            def ctx_drain(ci, upto, cap):
                done = 0
                while nk[ci] < min(TT, upto) and done < cap:
                    ctx_op(ci, nk[ci])
                    nk[ci] += 1
                    done += 1

            alloc_ctx(0)
            for tt in range(TT):
                scores_tt(1, tt)
                if tt < 6:
                    ctx_drain(0, TT, 3)
                if tt == 6:
                    normalize_a(0)
                if tt == 7:
                    normalize_b(0)
                if tt == 8:
                    alloc_ctx(1)
                if tt >= 8:
                    ctx_drain(1, tt, 2)
                if tt >= 8:
                    dt = tt - 8
                    if dt == 0:
                        qq_t2 = ps_cx.tile(
                            [P, CW], F32, tag="aux", bufs=2, name="qq2"
                        )
                    nc.tensor.matmul(
                        qq_t2[:],
                        wtiles["wq"][:, dt, :],
                        qq_pre[(2, dt)][:],
                        start=(dt == 0),
                        stop=(dt == KT_D - 1),
                    )
            nc.vector.tensor_copy(qt_sb[:, 2 * CW : 3 * CW], qq_t2[:])

            # ---------- chunk 2: ctx(2), qq3, outproj(0), AG(1) ----------
            nk[2] = 0
            for tt in range(TT):
                scores_tt(2, tt)
                if tt == 0:
                    ctx_drain(1, TT, 2)
                    op_dma(0)
                if tt == 1:
                    normalize_a(1)
                if tt == 3:
                    normalize_b(1)
                if tt == 4:
                    alloc_ctx(2)
                if tt >= 4:
                    ctx_drain(2, tt, 2)
                if tt < 8:
                    dt = tt
                    if dt == 0:
                        qq_t3 = ps_cx.tile(
                            [P, CW], F32, tag="aux", bufs=2, name="qq3"
                        )
                    nc.tensor.matmul(
                        qq_t3[:],
                        wtiles["wq"][:, dt, :],
                        qq_pre[(3, dt)][:],
                        start=(dt == 0),
                        stop=(dt == KT_D - 1),
                    )
                if tt == 8:
                    nc.vector.tensor_copy(qt_sb[:, 3 * CW : 4 * CW], qq_t3[:])

            # ---------- chunk 3: ctx(3), outproj(0+1), AG(2) ----------
            nk[3] = 0
            for tt in range(TT):
                scores_tt(3, tt)
                if tt == 0:
                    ctx_drain(2, TT, 2)
                    op_dma(1)
                if tt == 1:
                    normalize_a(2)
                if tt == 3:
                    normalize_b(2)
                if tt == 4:
                    alloc_ctx(3)
                if tt >= 4:
                    ctx_drain(3, tt, 2)
                if 2 <= tt <= 5:
                    op_mm(0, (tt - 2) * 2)
                    op_mm(0, (tt - 2) * 2 + 1)
                if tt == 6:
                    op_fin(0)
                if 12 <= tt <= 15:
                    op_mm(1, (tt - 12) * 2)
                    op_mm(1, (tt - 12) * 2 + 1)
            ctx_drain(3, TT, 4)
            op_fin(1)
            op_dma(2, split=True)
            normalize(3)

            # ---------- tail: outproj(2) overlaps AG(3), then outproj(3) --
            for kt in range(KT_D):
                op_mm(2, kt)
            op_fin(2)
            op_dma(3, split=True)
            wmk = ps_at.tile([P, 1024], F32, tag="mega", bufs=2, name="wmk")
            for _ in range(12):
                nc.tensor.matmul(
                    wmk[:, 0:NQ], warm_sb[:, 0:P], warm_sb[:],
                    start=True, stop=True,
                )
            for kt in range(KT_D):
                op_mm(3, kt)
            op_fin(3)

            ps_cx_cm.__exit__(None, None, None)
            ps_at_cm.__exit__(None, None, None)

    nc.compile()
    return nc


def kernel(
    encodings_for_q,
    encodings_for_k,
    encodings_for_v,
    W_q,
    W_k,
    W_v,
    W_out,
    _trace: bool = False,
):
    encodings_for_q = np.asarray(encodings_for_q, dtype=np.float32)
    encodings_for_k = np.asarray(encodings_for_k, dtype=np.float32)
    encodings_for_v = np.asarray(encodings_for_v, dtype=np.float32)
    W_q = np.asarray(W_q, dtype=np.float32)
    W_k = np.asarray(W_k, dtype=np.float32)
    W_v = np.asarray(W_v, dtype=np.float32)
    W_out = np.asarray(W_out, dtype=np.float32)

    if "nc" not in _cache:
        _cache["nc"] = build()
    nc = _cache["nc"]

    eqT = np.ascontiguousarray(encodings_for_q.T).astype(BF16NP)
    ekT = np.ascontiguousarray(encodings_for_k.T).astype(BF16NP)
    evT = np.ascontiguousarray(encodings_for_v.T).astype(BF16NP)

    in_maps = []
    for c in range(NCORES):
        hs = slice(HPC * c, HPC * (c + 1))
        in_maps.append(
            {
                "encq_t": eqT,
                "enck_t": ekT,
                "encv_t": evT,
                "wq": _prep_w(np.transpose(W_q[hs], (1, 0, 2)).reshape(D, FW)),
                "wk": _prep_w(np.transpose(W_k[hs], (1, 0, 2)).reshape(D, FW)),
                "wv": _prep_w(np.transpose(W_v[hs], (1, 0, 2)).reshape(D, FW)),
                "wo": _prep_w(W_out[:, FW * c : FW * (c + 1)]),
            }
        )

    r = run_bass_kernel_spmd(
        nc, in_maps, core_ids=list(range(NCORES)), trace=_trace
    )
    out = np.concatenate(
        [r.results[c]["outT"].T for c in range(NCORES)], axis=1
    )
    if _trace:
        kernel.last_exec_time_ns = r.exec_time_ns
        kernel.last_insts = (
            r.instructions_and_trace[0] if r.instructions_and_trace else None
        )
    return out.astype(np.float32)


# revision 18
# speedup vs baseline: 1.0315x; 1.0315x over previous
"""Multi-head attention (S=2048, D=1024, H=16, dk=dv=64) on 8 TRN2 NeuronCores.

Sharding: head-parallel tensor parallelism. Core c owns heads {2c, 2c+1}:
  - QT/KT [128, S] (two heads stacked on partitions); V via PE-transpose of
    VT, augmented with a ones column so the ctx matmul also produces the
    softmax denominators (softmax runs over the partition axis).
  - scoresT tiles -> exp on ACT (scale=1/8 folded in) -> ctx accumulation
    trailing the scores stream inside the same chunk.
  - per s-chunk: normalize ctxT at the chunk boundary (reciprocal chain on
    DVE, broadcast via a ones-stationary PE matmul into a mega PSUM slot),
    AllGather the [128, chunk] block -> [1024, chunk], then that chunk's
    128-row slice of the output projection interleaved into later chunks.

Queue discipline: ACT (scalar) queue only issues DMA during phase 0 (idle
before the first exp); all mid-stream DMAs ride sync; collectives own the
gpsimd queue; outproj gather loads are emitted before the NEXT collective
trigger so their completion wait covers only their own AllGather.

Compute dtype: bf16 operands (host-cast), fp32 PSUM, softmax in fp32.
"""

import ml_dtypes
import numpy as np

import concourse.bass as bass
import concourse.mybir as mybir
import concourse.tile as tile
from concourse import bacc
from concourse.bass_utils import run_bass_kernel_spmd

S = 2048
D = 1024
H = 16
DK = 64
DV = 64
NCORES = 8
HPC = H // NCORES          # heads per core = 2
FW = HPC * DV              # per-core feature width = 128
P = 128                    # partitions
KT_D = D // P              # 8 contraction tiles over D
TT = S // P                # 16 tiles over t (keys)
NQ = 512                   # matmul moving free dim
CW = 512                   # s-chunk width (ctx/AG granularity)
VA = 2 * (DV + 1)          # V_aug feature width

F32 = mybir.dt.float32
BF16 = mybir.dt.bfloat16
EXPF = mybir.ActivationFunctionType.Exp
BF16NP = ml_dtypes.bfloat16

_cache = {}


def _prep_w(w):
    """[D, FW] -> [128, KT_D, FW]: row p holds all d-tiles' row p."""
    return np.ascontiguousarray(
        np.transpose(w.reshape(KT_D, P, FW), (1, 0, 2))
    ).astype(BF16NP)


def build():
    nc = bacc.Bacc(None, target_bir_lowering=False)

    enc_in = {
        x: nc.dram_tensor(f"enc{x}_t", [D, S], BF16, kind="ExternalInput")
        for x in ("q", "k", "v")
    }
    w_in = {
        n: nc.dram_tensor(n, [P, KT_D, FW], BF16, kind="ExternalInput")
        for n in ("wq", "wk", "wv", "wo")
    }
    out_t = nc.dram_tensor("outT", [FW, S], F32, kind="ExternalOutput")

    from concourse.bass import _add_dep_helper
    from concourse.masks import make_identity

    with tile.TileContext(nc) as tc:
        with (
            tc.tile_pool(name="wts", bufs=1) as wts,
            tc.tile_pool(name="encp", bufs=3) as encp,
            tc.tile_pool(name="qkv", bufs=1) as qkv,
            tc.tile_pool(name="expp", bufs=13) as expp,
            tc.tile_pool(name="catp", bufs=1) as catp,
            tc.tile_pool(name="catin", bufs=16) as catin,
            tc.tile_pool(name="misc", bufs=1) as misc,
            tc.tile_pool(name="dram", bufs=1, space="DRAM") as dram,
        ):
            rg = [list(range(NCORES))]

            # ---- warmup fodder: memset then matmul garbage to ramp PE ----
            warm_sb = misc.tile([P, NQ], BF16, tag="warmsb")
            nc.vector.memset(warm_sb[:], 0.0)

            wtiles = {}

            def load_w(name, eng):
                wt = wts.tile([P, KT_D, FW], BF16, tag=f"w_{name}", name=name)
                eng.dma_start(wt[:], w_in[name][:])
                wtiles[name] = wt

            load_w("wk", nc.sync)
            load_w("wq", nc.scalar)

            ident = wts.tile([P, P], BF16, tag="ident")
            make_identity(nc, ident)
            ones_sb = wts.tile([1, DK], BF16, tag="ones")
            nc.vector.memset(ones_sb[:], 1.0)

            # persistent SBUF state
            qt_sb = qkv.tile([P, S], BF16, tag="qt")
            kt_sb = qkv.tile([P, S], BF16, tag="kt")
            vt_sb = qkv.tile([P, S], BF16, tag="vt")
            v_aug = qkv.tile([P, TT, VA], BF16, tag="vaug")
            cat_loc = catp.tile([P, S], BF16, tag="cat")
            nc.any.memset(v_aug[:, :, DV : DV + 1], 1.0)
            nc.any.memset(v_aug[:, :, 2 * DV + 1 : 2 * DV + 2], 1.0)

            # K tiles: halves on the two HWDGE queues, straight bf16
            def load_k(dt):
                t = encp.tile([P, S], BF16, tag="bfk", bufs=5, name="bf")
                nc.sync.dma_start(
                    t[:, :1024], enc_in["k"][dt * P : (dt + 1) * P, 0:1024]
                )
                nc.scalar.dma_start(
                    t[:, 1024:], enc_in["k"][dt * P : (dt + 1) * P, 1024:]
                )
                return t

            # Q quarter tiles [128, 512], alternating queues
            gate_inst = [None]

            def load_qq(qq, dt, engs=(None, None)):
                c0 = qq * CW
                t = encp.tile([P, CW], BF16, tag="bfq", bufs=18, name="bf")
                eng = engs[dt % 2] or (nc.sync if dt % 2 == 0 else nc.scalar)
                d = eng.dma_start(
                    t[:], enc_in["q"][dt * P : (dt + 1) * P, c0 : c0 + CW]
                )
                if qq == 1 and dt == 4:
                    gate_inst[0] = d.ins
                return t

            # ---- phase 0: K, Q quarters 0+1; PE warm-up burst ----
            ps_p_cm = tc.tile_pool(name="ps_p", bufs=1, space="PSUM")
            ps_p = ps_p_cm.__enter__()
            kacc = {
                sc4: ps_p.tile([P, NQ], F32, tag=f"ka{sc4}", name=f"ka{sc4}")
                for sc4 in range(4)
            }
            wm = ps_p.tile([P, NQ], F32, tag="warm", name="wm")
            for _ in range(14):
                nc.tensor.matmul(
                    wm[:], warm_sb[:, 0:P], warm_sb[:],
                    start=True, stop=True,
                )
            for dt in range(KT_D):
                ek = load_k(dt)
                for sc4 in range(4):
                    nc.tensor.matmul(
                        kacc[sc4][:],
                        wtiles["wk"][:, dt, :],
                        ek[:, sc4 * NQ : (sc4 + 1) * NQ],
                        start=(dt == 0),
                        stop=(dt == KT_D - 1),
                    )
            load_w("wv", nc.sync)
            load_w("wo", nc.scalar)
            for sc4 in range(4):
                nc.vector.tensor_copy(
                    kt_sb[:, sc4 * NQ : (sc4 + 1) * NQ], kacc[sc4][:]
                )
            qq01 = {
                qq: ps_p.tile([P, CW], F32, tag=f"qq{qq}", name=f"qq{qq}")
                for qq in range(2)
            }
            for qq in range(2):
                for dt in range(KT_D):
                    eq = load_qq(qq, dt)
                    nc.tensor.matmul(
                        qq01[qq][:],
                        wtiles["wq"][:, dt, :],
                        eq[:],
                        start=(dt == 0),
                        stop=(dt == KT_D - 1),
                    )
                nc.vector.tensor_copy(
                    qt_sb[:, qq * CW : (qq + 1) * CW], qq01[qq][:]
                )
            ps_p_cm.__exit__(None, None, None)

            # ---- enc_v: SWDGE stream, held behind the K/Q stream ----
            ev_tiles = []
            for dt in range(KT_D):
                ev = encp.tile([P, S], BF16, tag="encv", bufs=6, name="ev")
                d = nc.gpsimd.dma_start(
                    ev[:], enc_in["v"][dt * P : (dt + 1) * P, :]
                )
                if dt == 0 and gate_inst[0] is not None:
                    _add_dep_helper(d.ins, gate_inst[0], sync=True,
                                    reason="defer enc_v behind K/Q stream")
                ev_tiles.append(ev)

            qq_pre = {}
            for qq in (2, 3):
                for dt in range(KT_D):
                    qq_pre[(qq, dt)] = load_qq(
                        qq, dt, engs=(nc.sync, nc.gpsimd)
                    )

            # ---- attention stream ----
            ps_at_cm = tc.tile_pool(name="ps_at", bufs=1, space="PSUM")
            ps_at = ps_at_cm.__enter__()
            ctx_ps = {}
            gas = {}
            exs = {}

            def scores_tt(ci, tt):
                m = ps_at.tile([P, 1024], F32, tag="mega", bufs=2, name="m")
                s0 = ci * CW
                for h in range(HPC):
                    nc.tensor.matmul(
                        m[:, h * NQ : (h + 1) * NQ],
                        kt_sb[h * DK : (h + 1) * DK, tt * P : (tt + 1) * P],
                        qt_sb[h * DK : (h + 1) * DK, s0 : s0 + NQ],
                        start=True,
                        stop=True,
                    )
                ex = expp.tile(
                    [P, 1024], BF16, tag=f"exp{tt % 2}", bufs=13, name="ex"
                )
                nc.scalar.activation(ex[:], m[:], EXPF, scale=1.0 / np.sqrt(DK))
                exs[(ci, tt)] = ex

            def ctx_op(ci, k):
                for h in range(HPC):
                    nc.tensor.matmul(
                        ctx_ps[(ci, h)][:, :],
                        v_aug[:, k, h * (DV + 1) : (h + 1) * (DV + 1)],
                        exs[(ci, k)][:, h * NQ : (h + 1) * NQ],
                        start=(k == 0),
                        stop=(k == TT - 1),
                    )

            def alloc_ctx(ci):
                for h in range(HPC):
                    ctx_ps[(ci, h)] = ps_cx.tile(
                        [DV + 1, CW], F32, tag=f"cx{h}", bufs=1,
                        name=f"cx{ci}{h}",
                    )

            recipbs = {}

            def normalize_a(ci):
                recs = []
                for h in range(HPC):
                    den = misc.tile([1, CW], F32, tag="den", bufs=2, name="den")
                    nc.vector.tensor_copy(
                        den[:], ctx_ps[(ci, h)][DV : DV + 1, :]
                    )
                    recip = misc.tile(
                        [1, CW], F32, tag="recip", bufs=2, name="recip"
                    )
                    nc.vector.reciprocal_approx_fast(recip[:], den[:])
                    recipb = misc.tile(
                        [1, CW], BF16, tag="recipb", bufs=2, name="recipb"
                    )
                    nc.vector.tensor_copy(recipb[:], recip[:])
                    recs.append(recipb)
                recipbs[ci] = recs

            def normalize_b(ci):
                c0 = ci * CW
                bc = ps_at.tile([P, 1024], F32, tag="mega", bufs=2, name="bc")
                for h in range(HPC):
                    nc.tensor.matmul(
                        bc[h * DV : (h + 1) * DV, 0:CW],
                        ones_sb[:, 0:DV],
                        recipbs[ci][h][:],
                        start=True,
                        stop=True,
                    )
                bsb = misc.tile([P, CW], F32, tag="bsb", bufs=2, name="bsb")
                nc.vector.tensor_copy(bsb[:], bc[:, 0:CW])
                for h in range(HPC):
                    nc.vector.tensor_mul(
                        cat_loc[h * DV : (h + 1) * DV, c0 : c0 + CW],
                        ctx_ps[(ci, h)][0:DV, :],
                        bsb[h * DV : (h + 1) * DV, :],
                    )
                cb = dram.tile([P, CW], BF16, tag=f"catb{ci}", name="cb")
                nc.sync.dma_start(cb[:], cat_loc[:, c0 : c0 + CW])
                ga = dram.tile([D, CW], BF16, tag=f"catall{ci}", name="ga")
                nc.gpsimd.collective_compute(
                    "AllGather",
                    mybir.AluOpType.bypass,
                    ins=[cb[:].opt()],
                    outs=[ga[:].opt()],
                    replica_groups=rg,
                )
                gas[ci] = ga

            def normalize(ci):
                normalize_a(ci)
                normalize_b(ci)

            # interleaved outproj pieces; op_dma(ci) must be emitted BEFORE
            # the NEXT chunk's collective trigger so its completion wait
            # covers only AllGather(ci).
            om_ps = {}
            op_ct = {}

            def op_dma(ci):
                for kt in range(KT_D):
                    ct = catin.tile([P, CW], BF16, tag="catkt", name="ct")
                    nc.sync.dma_start(
                        ct[:], gas[ci][kt * P : (kt + 1) * P, :]
                    )
                    op_ct[(ci, kt)] = ct

            def op_mm(ci, kt):
                if kt == 0:
                    om_ps[ci] = ps_cx.tile(
                        [P, CW], F32, tag="aux", bufs=2, name=f"om{ci}"
                    )
                nc.tensor.matmul(
                    om_ps[ci][:],
                    wtiles["wo"][:, kt, :],
                    op_ct[(ci, kt)][:],
                    start=(kt == 0),
                    stop=(kt == KT_D - 1),
                )

            def op_fin(ci):
                c0 = ci * CW
                ob = misc.tile([P, CW], F32, tag="ob", bufs=2, name="ob")
                nc.vector.tensor_copy(ob[:], om_ps[ci][:])
                nc.sync.dma_start(out_t[:, c0 : c0 + CW], ob[:])

            # ---------- chunk 0: scores + V proj + transposes ----------
            ps_v2_cm = tc.tile_pool(name="ps_v2", bufs=1, space="PSUM")
            ps_v2 = ps_v2_cm.__enter__()
            vacc = {
                half: ps_v2.tile(
                    [P, 1024], F32, tag=f"va{half}", name=f"va{half}"
                )
                for half in range(2)
            }
            ps_cx = None
            ps_cx_cm = None
            tr_k = [0]

            def transposes(n):
                for _ in range(n):
                    k = tr_k[0]
                    if k >= TT:
                        return
                    tr_k[0] += 1
                    tp = ps_cx.tile(
                        [P, P], BF16, tag="aux", bufs=2, name="tp"
                    )
                    nc.tensor.transpose(
                        tp[:], vt_sb[:, k * P : (k + 1) * P], ident[:]
                    )
                    nc.vector.tensor_copy(v_aug[:, k, 0:DV], tp[:, 0:DV])
                    nc.vector.tensor_copy(
                        v_aug[:, k, DV + 1 : 2 * DV + 1],
                        tp[:, DV : 2 * DV],
                    )

            for tt in range(TT):
                scores_tt(0, tt)
                if 2 <= tt <= 9:
                    dt = tt - 2
                    for half in range(2):
                        for nn in range(2):
                            off = half * 1024 + nn * NQ
                            nc.tensor.matmul(
                                vacc[half][:, nn * NQ : (nn + 1) * NQ],
                                wtiles["wv"][:, dt, :],
                                ev_tiles[dt][:, off : off + NQ],
                                start=(dt == 0),
                                stop=(dt == KT_D - 1),
                            )
                if tt == 10:
                    for nn in range(4):
                        nc.vector.tensor_copy(
                            vt_sb[:, nn * NQ : (nn + 1) * NQ],
                            vacc[nn // 2][:, (nn % 2) * NQ : (nn % 2 + 1) * NQ],
                        )
                    ps_v2_cm.__exit__(None, None, None)
                    ps_cx_cm = tc.tile_pool(name="ps_cx", bufs=1, space="PSUM")
                    ps_cx = ps_cx_cm.__enter__()
                if tt >= 10:
                    transposes(3)

            # ---------- chunk 1: ctx(0) drain, AG(0), ctx(1), qq2 ----------
            nk = {0: 0, 1: 0}

            def ctx_drain(ci, upto, cap):
                done = 0
                while nk[ci] < min(TT, upto) and done < cap:
                    ctx_op(ci, nk[ci])
                    nk[ci] += 1
                    done += 1

            alloc_ctx(0)
            for tt in range(TT):
                scores_tt(1, tt)
                if tt < 6:
                    ctx_drain(0, TT, 3)
                if tt == 6:
                    normalize_a(0)
                if tt == 7:
                    normalize_b(0)
                if tt == 8:
                    alloc_ctx(1)
                if tt >= 8:
                    ctx_drain(1, tt, 2)
                if tt >= 8:
                    dt = tt - 8
                    if dt == 0:
                        qq_t2 = ps_cx.tile(
                            [P, CW], F32, tag="aux", bufs=2, name="qq2"
                        )
                    nc.tensor.matmul(
                        qq_t2[:],
                        wtiles["wq"][:, dt, :],
                        qq_pre[(2, dt)][:],
                        start=(dt == 0),
                        stop=(dt == KT_D - 1),
                    )
            nc.vector.tensor_copy(qt_sb[:, 2 * CW : 3 * CW], qq_t2[:])

            # ---------- chunk 2: ctx(2), qq3, outproj(0), AG(1) ----------
            nk[2] = 0
            for tt in range(TT):
                scores_tt(2, tt)
                if tt == 0:
                    ctx_drain(1, TT, 2)
                    op_dma(0)
                if tt == 1:
                    normalize_a(1)
                if tt == 3:
                    normalize_b(1)
                if tt == 4:
                    alloc_ctx(2)
                if tt >= 4:
                    ctx_drain(2, tt, 2)
                if tt < 8:
                    dt = tt
                    if dt == 0:
                        qq_t3 = ps_cx.tile(
                            [P, CW], F32, tag="aux", bufs=2, name="qq3"
                        )
                    nc.tensor.matmul(
                        qq_t3[:],
                        wtiles["wq"][:, dt, :],
                        qq_pre[(3, dt)][:],
                        start=(dt == 0),
                        stop=(dt == KT_D - 1),
                    )
                if tt == 8:
                    nc.vector.tensor_copy(qt_sb[:, 3 * CW : 4 * CW], qq_t3[:])

            # ---------- chunk 3: ctx(3), outproj(0+1), AG(2) ----------
            nk[3] = 0
            for tt in range(TT):
                scores_tt(3, tt)
                if tt == 0:
                    ctx_drain(2, TT, 2)
                    op_dma(1)
                if tt == 1:
                    normalize_a(2)
                if tt == 3:
                    normalize_b(2)
                if tt == 4:
                    alloc_ctx(3)
                if tt >= 4:
                    ctx_drain(3, tt, 2)
                if 2 <= tt <= 5:
                    op_mm(0, (tt - 2) * 2)
                    op_mm(0, (tt - 2) * 2 + 1)
                if tt == 6:
                    op_fin(0)
                if 12 <= tt <= 15:
                    op_mm(1, (tt - 12) * 2)
                    op_mm(1, (tt - 12) * 2 + 1)
            ctx_drain(3, TT, 4)
            op_fin(1)
            op_dma(2)
            normalize(3)

            # ---------- tail: outproj(2) overlaps AG(3), then outproj(3) --
            for kt in range(KT_D):
                op_mm(2, kt)
            op_fin(2)
            op_dma(3)
            for kt in range(KT_D):
                op_mm(3, kt)
            op_fin(3)

            ps_cx_cm.__exit__(None, None, None)
            ps_at_cm.__exit__(None, None, None)

    nc.compile()
    return nc


def kernel(
    encodings_for_q,
    encodings_for_k,
    encodings_for_v,
    W_q,
    W_k,
    W_v,
    W_out,
    _trace: bool = False,
):
    encodings_for_q = np.asarray(encodings_for_q, dtype=np.float32)
    encodings_for_k = np.asarray(encodings_for_k, dtype=np.float32)
    encodings_for_v = np.asarray(encodings_for_v, dtype=np.float32)
    W_q = np.asarray(W_q, dtype=np.float32)
    W_k = np.asarray(W_k, dtype=np.float32)
    W_v = np.asarray(W_v, dtype=np.float32)
    W_out = np.asarray(W_out, dtype=np.float32)

    if "nc" not in _cache:
        _cache["nc"] = build()
    nc = _cache["nc"]

    eqT = np.ascontiguousarray(encodings_for_q.T).astype(BF16NP)
    ekT = np.ascontiguousarray(encodings_for_k.T).astype(BF16NP)
    evT = np.ascontiguousarray(encodings_for_v.T).astype(BF16NP)

    in_maps = []
    for c in range(NCORES):
        hs = slice(HPC * c, HPC * (c + 1))
        in_maps.append(
            {
                "encq_t": eqT,
                "enck_t": ekT,
                "encv_t": evT,
                "wq": _prep_w(np.transpose(W_q[hs], (1, 0, 2)).reshape(D, FW)),
                "wk": _prep_w(np.transpose(W_k[hs], (1, 0, 2)).reshape(D, FW)),
                "wv": _prep_w(np.transpose(W_v[hs], (1, 0, 2)).reshape(D, FW)),
                "wo": _prep_w(W_out[:, FW * c : FW * (c + 1)]),
            }
        )

    r = run_bass_kernel_spmd(
        nc, in_maps, core_ids=list(range(NCORES)), trace=_trace
    )
    out = np.concatenate(
        [r.results[c]["outT"].T for c in range(NCORES)], axis=1
    )
    if _trace:
        kernel.last_exec_time_ns = r.exec_time_ns
        kernel.last_insts = (
            r.instructions_and_trace[0] if r.instructions_and_trace else None
        )
    return out.astype(np.float32)


# revision 19
# speedup vs baseline: 1.0502x; 1.0181x over previous
"""Multi-head attention (S=2048, D=1024, H=16, dk=dv=64) on 8 TRN2 NeuronCores.

Sharding: head-parallel tensor parallelism. Core c owns heads {2c, 2c+1}:
  - QT/KT [128, S] (two heads stacked on partitions); V via PE-transpose of
    VT, augmented with a ones column so the ctx matmul also produces the
    softmax denominators (softmax runs over the partition axis).
  - scoresT tiles -> exp on ACT (scale=1/8 folded in) -> ctx accumulation
    trailing the scores stream inside the same chunk.
  - per s-chunk: normalize ctxT at the chunk boundary (reciprocal chain on
    DVE, broadcast via a ones-stationary PE matmul into a mega PSUM slot),
    AllGather the [128, chunk] block -> [1024, chunk], then that chunk's
    128-row slice of the output projection interleaved into later chunks.

Queue discipline: ACT (scalar) queue only issues DMA during phase 0 (idle
before the first exp); all mid-stream DMAs ride sync; collectives own the
gpsimd queue; outproj gather loads are emitted before the NEXT collective
trigger so their completion wait covers only their own AllGather.

Compute dtype: bf16 operands (host-cast), fp32 PSUM, softmax in fp32.
"""

import ml_dtypes
import numpy as np

import concourse.bass as bass
import concourse.mybir as mybir
import concourse.tile as tile
from concourse import bacc
from concourse.bass_utils import run_bass_kernel_spmd

S = 2048
D = 1024
H = 16
DK = 64
DV = 64
NCORES = 8
HPC = H // NCORES          # heads per core = 2
FW = HPC * DV              # per-core feature width = 128
P = 128                    # partitions
KT_D = D // P              # 8 contraction tiles over D
TT = S // P                # 16 tiles over t (keys)
NQ = 512                   # matmul moving free dim
CW = 512                   # s-chunk width (ctx/AG granularity)
VA = 2 * (DV + 1)          # V_aug feature width

F32 = mybir.dt.float32
BF16 = mybir.dt.bfloat16
EXPF = mybir.ActivationFunctionType.Exp
BF16NP = ml_dtypes.bfloat16

_cache = {}


def _prep_w(w):
    """[D, FW] -> [128, KT_D, FW]: row p holds all d-tiles' row p."""
    return np.ascontiguousarray(
        np.transpose(w.reshape(KT_D, P, FW), (1, 0, 2))
    ).astype(BF16NP)


def build():
    nc = bacc.Bacc(None, target_bir_lowering=False)

    enc_in = {
        x: nc.dram_tensor(f"enc{x}_t", [D, S], BF16, kind="ExternalInput")
        for x in ("q", "k", "v")
    }
    w_in = {
        n: nc.dram_tensor(n, [P, KT_D, FW], BF16, kind="ExternalInput")
        for n in ("wq", "wk", "wv", "wo")
    }
    out_t = nc.dram_tensor("outT", [FW, S], F32, kind="ExternalOutput")

    from concourse.bass import _add_dep_helper
    from concourse.masks import make_identity

    with tile.TileContext(nc) as tc:
        with (
            tc.tile_pool(name="wts", bufs=1) as wts,
            tc.tile_pool(name="encp", bufs=3) as encp,
            tc.tile_pool(name="qkv", bufs=1) as qkv,
            tc.tile_pool(name="expp", bufs=13) as expp,
            tc.tile_pool(name="catp", bufs=1) as catp,
            tc.tile_pool(name="catin", bufs=16) as catin,
            tc.tile_pool(name="misc", bufs=1) as misc,
            tc.tile_pool(name="dram", bufs=1, space="DRAM") as dram,
        ):
            rg = [list(range(NCORES))]

            # ---- warmup fodder: memset then matmul garbage to ramp PE ----
            warm_sb = misc.tile([P, NQ], BF16, tag="warmsb")
            nc.vector.memset(warm_sb[:], 0.0)

            wtiles = {}

            def load_w(name, eng):
                wt = wts.tile([P, KT_D, FW], BF16, tag=f"w_{name}", name=name)
                eng.dma_start(wt[:], w_in[name][:])
                wtiles[name] = wt

            load_w("wk", nc.sync)
            load_w("wq", nc.scalar)

            ident = wts.tile([P, P], BF16, tag="ident")
            make_identity(nc, ident)
            ones_sb = wts.tile([1, DK], BF16, tag="ones")
            nc.vector.memset(ones_sb[:], 1.0)

            # persistent SBUF state
            qt_sb = qkv.tile([P, S], BF16, tag="qt")
            kt_sb = qkv.tile([P, S], BF16, tag="kt")
            vt_sb = qkv.tile([P, S], BF16, tag="vt")
            v_aug = qkv.tile([P, TT, VA], BF16, tag="vaug")
            cat_loc = catp.tile([P, S], BF16, tag="cat")
            nc.any.memset(v_aug[:, :, DV : DV + 1], 1.0)
            nc.any.memset(v_aug[:, :, 2 * DV + 1 : 2 * DV + 2], 1.0)

            # K tiles: halves on the two HWDGE queues, straight bf16
            def load_k(dt):
                t = encp.tile([P, S], BF16, tag="bfk", bufs=5, name="bf")
                nc.sync.dma_start(
                    t[:, :1024], enc_in["k"][dt * P : (dt + 1) * P, 0:1024]
                )
                nc.scalar.dma_start(
                    t[:, 1024:], enc_in["k"][dt * P : (dt + 1) * P, 1024:]
                )
                return t

            # Q quarter tiles [128, 512], alternating queues
            gate_inst = [None]

            def load_qq(qq, dt, engs=(None, None)):
                c0 = qq * CW
                t = encp.tile([P, CW], BF16, tag="bfq", bufs=18, name="bf")
                eng = engs[dt % 2] or (nc.sync if dt % 2 == 0 else nc.scalar)
                d = eng.dma_start(
                    t[:], enc_in["q"][dt * P : (dt + 1) * P, c0 : c0 + CW]
                )
                if qq == 1 and dt == 4:
                    gate_inst[0] = d.ins
                return t

            # ---- phase 0: K, Q quarters 0+1; PE warm-up burst ----
            ps_p_cm = tc.tile_pool(name="ps_p", bufs=1, space="PSUM")
            ps_p = ps_p_cm.__enter__()
            kacc = {
                sc4: ps_p.tile([P, NQ], F32, tag=f"ka{sc4}", name=f"ka{sc4}")
                for sc4 in range(4)
            }
            wm = ps_p.tile([P, NQ], F32, tag="warm", name="wm")
            for _ in range(14):
                nc.tensor.matmul(
                    wm[:], warm_sb[:, 0:P], warm_sb[:],
                    start=True, stop=True,
                )
            for dt in range(KT_D):
                ek = load_k(dt)
                for sc4 in range(4):
                    nc.tensor.matmul(
                        kacc[sc4][:],
                        wtiles["wk"][:, dt, :],
                        ek[:, sc4 * NQ : (sc4 + 1) * NQ],
                        start=(dt == 0),
                        stop=(dt == KT_D - 1),
                    )
            load_w("wv", nc.sync)
            load_w("wo", nc.scalar)
            for sc4 in range(4):
                nc.vector.tensor_copy(
                    kt_sb[:, sc4 * NQ : (sc4 + 1) * NQ], kacc[sc4][:]
                )
            qq01 = {
                qq: ps_p.tile([P, CW], F32, tag=f"qq{qq}", name=f"qq{qq}")
                for qq in range(2)
            }
            for qq in range(2):
                for dt in range(KT_D):
                    eq = load_qq(qq, dt)
                    nc.tensor.matmul(
                        qq01[qq][:],
                        wtiles["wq"][:, dt, :],
                        eq[:],
                        start=(dt == 0),
                        stop=(dt == KT_D - 1),
                    )
                nc.vector.tensor_copy(
                    qt_sb[:, qq * CW : (qq + 1) * CW], qq01[qq][:]
                )
            ps_p_cm.__exit__(None, None, None)

            # ---- enc_v: SWDGE stream, held behind the K/Q stream ----
            ev_tiles = []
            for dt in range(KT_D):
                ev = encp.tile([P, S], BF16, tag="encv", bufs=6, name="ev")
                d = nc.gpsimd.dma_start(
                    ev[:], enc_in["v"][dt * P : (dt + 1) * P, :]
                )
                if dt == 0 and gate_inst[0] is not None:
                    _add_dep_helper(d.ins, gate_inst[0], sync=True,
                                    reason="defer enc_v behind K/Q stream")
                ev_tiles.append(ev)

            qq_pre = {}
            for qq in (2, 3):
                for dt in range(KT_D):
                    qq_pre[(qq, dt)] = load_qq(
                        qq, dt, engs=(nc.sync, nc.gpsimd)
                    )

            # ---- attention stream ----
            ps_at_cm = tc.tile_pool(name="ps_at", bufs=1, space="PSUM")
            ps_at = ps_at_cm.__enter__()
            ctx_ps = {}
            gas = {}
            exs = {}

            def scores_tt(ci, tt):
                m = ps_at.tile([P, 1024], F32, tag="mega", bufs=2, name="m")
                s0 = ci * CW
                for h in range(HPC):
                    nc.tensor.matmul(
                        m[:, h * NQ : (h + 1) * NQ],
                        kt_sb[h * DK : (h + 1) * DK, tt * P : (tt + 1) * P],
                        qt_sb[h * DK : (h + 1) * DK, s0 : s0 + NQ],
                        start=True,
                        stop=True,
                    )
                ex = expp.tile(
                    [P, 1024], BF16, tag=f"exp{tt % 2}", bufs=13, name="ex"
                )
                nc.scalar.activation(ex[:], m[:], EXPF, scale=1.0 / np.sqrt(DK))
                exs[(ci, tt)] = ex

            def ctx_op(ci, k):
                for h in range(HPC):
                    nc.tensor.matmul(
                        ctx_ps[(ci, h)][:, :],
                        v_aug[:, k, h * (DV + 1) : (h + 1) * (DV + 1)],
                        exs[(ci, k)][:, h * NQ : (h + 1) * NQ],
                        start=(k == 0),
                        stop=(k == TT - 1),
                    )

            def alloc_ctx(ci):
                for h in range(HPC):
                    ctx_ps[(ci, h)] = ps_cx.tile(
                        [DV + 1, CW], F32, tag=f"cx{h}", bufs=1,
                        name=f"cx{ci}{h}",
                    )

            recipbs = {}

            def normalize_a(ci):
                recs = []
                for h in range(HPC):
                    den = misc.tile([1, CW], F32, tag="den", bufs=2, name="den")
                    nc.vector.tensor_copy(
                        den[:], ctx_ps[(ci, h)][DV : DV + 1, :]
                    )
                    recip = misc.tile(
                        [1, CW], F32, tag="recip", bufs=2, name="recip"
                    )
                    nc.vector.reciprocal_approx_fast(recip[:], den[:])
                    recipb = misc.tile(
                        [1, CW], BF16, tag="recipb", bufs=2, name="recipb"
                    )
                    nc.vector.tensor_copy(recipb[:], recip[:])
                    recs.append(recipb)
                recipbs[ci] = recs

            def normalize_b(ci):
                c0 = ci * CW
                bc = ps_at.tile([P, 1024], F32, tag="mega", bufs=2, name="bc")
                for h in range(HPC):
                    nc.tensor.matmul(
                        bc[h * DV : (h + 1) * DV, 0:CW],
                        ones_sb[:, 0:DV],
                        recipbs[ci][h][:],
                        start=True,
                        stop=True,
                    )
                bsb = misc.tile([P, CW], F32, tag="bsb", bufs=2, name="bsb")
                nc.vector.tensor_copy(bsb[:], bc[:, 0:CW])
                for h in range(HPC):
                    nc.vector.tensor_mul(
                        cat_loc[h * DV : (h + 1) * DV, c0 : c0 + CW],
                        ctx_ps[(ci, h)][0:DV, :],
                        bsb[h * DV : (h + 1) * DV, :],
                    )
                cb = dram.tile([P, CW], BF16, tag=f"catb{ci}", name="cb")
                nc.sync.dma_start(cb[:], cat_loc[:, c0 : c0 + CW])
                ga = dram.tile([D, CW], BF16, tag=f"catall{ci}", name="ga")
                nc.gpsimd.collective_compute(
                    "AllGather",
                    mybir.AluOpType.bypass,
                    ins=[cb[:].opt()],
                    outs=[ga[:].opt()],
                    replica_groups=rg,
                )
                gas[ci] = ga

            def normalize(ci):
                normalize_a(ci)
                normalize_b(ci)

            # interleaved outproj pieces; op_dma(ci) must be emitted BEFORE
            # the NEXT chunk's collective trigger so its completion wait
            # covers only AllGather(ci).
            om_ps = {}
            op_ct = {}

            def op_dma(ci):
                for kt in range(KT_D):
                    ct = catin.tile([P, CW], BF16, tag="catkt", name="ct")
                    nc.sync.dma_start(
                        ct[:], gas[ci][kt * P : (kt + 1) * P, :]
                    )
                    op_ct[(ci, kt)] = ct

            def op_mm(ci, kt):
                if kt == 0:
                    om_ps[ci] = ps_cx.tile(
                        [P, CW], F32, tag="aux", bufs=2, name=f"om{ci}"
                    )
                nc.tensor.matmul(
                    om_ps[ci][:],
                    wtiles["wo"][:, kt, :],
                    op_ct[(ci, kt)][:],
                    start=(kt == 0),
                    stop=(kt == KT_D - 1),
                )

            def op_fin(ci):
                c0 = ci * CW
                ob = misc.tile([P, CW], F32, tag="ob", bufs=2, name="ob")
                nc.vector.tensor_copy(ob[:], om_ps[ci][:])
                nc.sync.dma_start(out_t[:, c0 : c0 + CW], ob[:])

            # ---------- chunk 0: scores + V proj + transposes ----------
            ps_v2_cm = tc.tile_pool(name="ps_v2", bufs=1, space="PSUM")
            ps_v2 = ps_v2_cm.__enter__()
            vacc = {
                half: ps_v2.tile(
                    [P, 1024], F32, tag=f"va{half}", name=f"va{half}"
                )
                for half in range(2)
            }
            ps_cx = None
            ps_cx_cm = None
            tr_k = [0]

            def transposes(n):
                for _ in range(n):
                    k = tr_k[0]
                    if k >= TT:
                        return
                    tr_k[0] += 1
                    tp = ps_cx.tile(
                        [P, P], BF16, tag="aux", bufs=2, name="tp"
                    )
                    nc.tensor.transpose(
                        tp[:], vt_sb[:, k * P : (k + 1) * P], ident[:]
                    )
                    nc.vector.tensor_copy(v_aug[:, k, 0:DV], tp[:, 0:DV])
                    nc.vector.tensor_copy(
                        v_aug[:, k, DV + 1 : 2 * DV + 1],
                        tp[:, DV : 2 * DV],
                    )

            for tt in range(TT):
                scores_tt(0, tt)
                if 2 <= tt <= 9:
                    dt = tt - 2
                    for half in range(2):
                        for nn in range(2):
                            off = half * 1024 + nn * NQ
                            nc.tensor.matmul(
                                vacc[half][:, nn * NQ : (nn + 1) * NQ],
                                wtiles["wv"][:, dt, :],
                                ev_tiles[dt][:, off : off + NQ],
                                start=(dt == 0),
                                stop=(dt == KT_D - 1),
                            )
                if tt == 10:
                    for nn in range(4):
                        nc.vector.tensor_copy(
                            vt_sb[:, nn * NQ : (nn + 1) * NQ],
                            vacc[nn // 2][:, (nn % 2) * NQ : (nn % 2 + 1) * NQ],
                        )
                    ps_v2_cm.__exit__(None, None, None)
                    ps_cx_cm = tc.tile_pool(name="ps_cx", bufs=1, space="PSUM")
                    ps_cx = ps_cx_cm.__enter__()
                if tt >= 10:
                    transposes(3)

            # ---------- chunk 1: ctx(0) drain, AG(0), ctx(1), qq2 ----------
            nk = {0: 0, 1: 0}

            def ctx_drain(ci, upto, cap):
                done = 0
                while nk[ci] < min(TT, upto) and done < cap:
                    ctx_op(ci, nk[ci])
                    nk[ci] += 1
                    done += 1

            alloc_ctx(0)
            for tt in range(TT):
                scores_tt(1, tt)
                if tt < 6:
                    ctx_drain(0, TT, 3)
                if tt == 6:
                    normalize_a(0)
                if tt == 7:
                    normalize_b(0)
                if tt == 8:
                    alloc_ctx(1)
                if tt >= 8:
                    ctx_drain(1, tt, 1)
                if tt >= 8:
                    dt = tt - 8
                    if dt == 0:
                        qq_t2 = ps_cx.tile(
                            [P, CW], F32, tag="aux", bufs=2, name="qq2"
                        )
                    nc.tensor.matmul(
                        qq_t2[:],
                        wtiles["wq"][:, dt, :],
                        qq_pre[(2, dt)][:],
                        start=(dt == 0),
                        stop=(dt == KT_D - 1),
                    )
            nc.vector.tensor_copy(qt_sb[:, 2 * CW : 3 * CW], qq_t2[:])

            # ---------- chunk 2: ctx(2), qq3, outproj(0), AG(1) ----------
            nk[2] = 0
            for tt in range(TT):
                scores_tt(2, tt)
                if tt == 0:
                    ctx_drain(1, TT, 16)
                    op_dma(0)
                if tt == 1:
                    normalize_a(1)
                if tt == 3:
                    normalize_b(1)
                if tt == 4:
                    alloc_ctx(2)
                if tt >= 8:
                    ctx_drain(2, tt, 1)
                if tt < 8:
                    dt = tt
                    if dt == 0:
                        qq_t3 = ps_cx.tile(
                            [P, CW], F32, tag="aux", bufs=2, name="qq3"
                        )
                    nc.tensor.matmul(
                        qq_t3[:],
                        wtiles["wq"][:, dt, :],
                        qq_pre[(3, dt)][:],
                        start=(dt == 0),
                        stop=(dt == KT_D - 1),
                    )
                if tt == 8:
                    nc.vector.tensor_copy(qt_sb[:, 3 * CW : 4 * CW], qq_t3[:])

            # ---------- chunk 3: ctx(3), outproj(0+1), AG(2) ----------
            nk[3] = 0
            for tt in range(TT):
                scores_tt(3, tt)
                if tt == 0:
                    ctx_drain(2, TT, 16)
                    op_dma(1)
                if tt == 1:
                    normalize_a(2)
                if tt == 3:
                    normalize_b(2)
                if tt == 4:
                    alloc_ctx(3)
                if tt >= 4:
                    ctx_drain(3, tt, 2)
                if 2 <= tt <= 5:
                    op_mm(0, (tt - 2) * 2)
                    op_mm(0, (tt - 2) * 2 + 1)
                if tt == 6:
                    op_fin(0)
                if 12 <= tt <= 15:
                    op_mm(1, (tt - 12) * 2)
                    op_mm(1, (tt - 12) * 2 + 1)
            ctx_drain(3, TT, 4)
            op_fin(1)
            op_dma(2)
            normalize(3)

            # ---------- tail: outproj(2) overlaps AG(3), then outproj(3) --
            for kt in range(KT_D):
                op_mm(2, kt)
            op_fin(2)
            op_dma(3)
            for kt in range(KT_D):
                op_mm(3, kt)
            op_fin(3)

            ps_cx_cm.__exit__(None, None, None)
            ps_at_cm.__exit__(None, None, None)

    nc.compile()
    return nc


def kernel(
    encodings_for_q,
    encodings_for_k,
    encodings_for_v,
    W_q,
    W_k,
    W_v,
    W_out,
    _trace: bool = False,
):
    encodings_for_q = np.asarray(encodings_for_q, dtype=np.float32)
    encodings_for_k = np.asarray(encodings_for_k, dtype=np.float32)
    encodings_for_v = np.asarray(encodings_for_v, dtype=np.float32)
    W_q = np.asarray(W_q, dtype=np.float32)
    W_k = np.asarray(W_k, dtype=np.float32)
    W_v = np.asarray(W_v, dtype=np.float32)
    W_out = np.asarray(W_out, dtype=np.float32)

    if "nc" not in _cache:
        _cache["nc"] = build()
    nc = _cache["nc"]

    eqT = np.ascontiguousarray(encodings_for_q.T).astype(BF16NP)
    ekT = np.ascontiguousarray(encodings_for_k.T).astype(BF16NP)
    evT = np.ascontiguousarray(encodings_for_v.T).astype(BF16NP)

    in_maps = []
    for c in range(NCORES):
        hs = slice(HPC * c, HPC * (c + 1))
        in_maps.append(
            {
                "encq_t": eqT,
                "enck_t": ekT,
                "encv_t": evT,
                "wq": _prep_w(np.transpose(W_q[hs], (1, 0, 2)).reshape(D, FW)),
                "wk": _prep_w(np.transpose(W_k[hs], (1, 0, 2)).reshape(D, FW)),
                "wv": _prep_w(np.transpose(W_v[hs], (1, 0, 2)).reshape(D, FW)),
                "wo": _prep_w(W_out[:, FW * c : FW * (c + 1)]),
            }
        )

    r = run_bass_kernel_spmd(
        nc, in_maps, core_ids=list(range(NCORES)), trace=_trace
    )
    out = np.concatenate(
        [r.results[c]["outT"].T for c in range(NCORES)], axis=1
    )
    if _trace:
        kernel.last_exec_time_ns = r.exec_time_ns
        kernel.last_insts = (
            r.instructions_and_trace[0] if r.instructions_and_trace else None
        )
    return out.astype(np.float32)
